# revision 18
# baseline (speedup 1.0000x reference)
"""Block-causal (key-block >= query-block) multi-head attention for
Trainium2, run SPMD on 8 NeuronCores.

Problem (hardcoded): B=2, T=8, N=256 (L=2048), D=768, H=12, HD=64.
reference:
    qkv = x @ qkv_w.T + qkv_b ; split into q,k,v heads
    s   = (q @ k.T) / 8 ; mask: query in block ti attends keys in blocks tj >= ti
    p   = softmax(s) ; y = p @ v ; out = y @ proj_w.T + proj_b

Sharding: data-parallel over B (2) x tensor-parallel over heads (4 groups of
3 heads) = 8 cores. Each core computes, for its (batch, head-group):
  - V natural  = x-chunk @ Wv^T    (per 128-key chunk, xT as PE stationary)
  - Q^T,K^T    = Wsel @ x^T        (3 chunks of 128: [q0|q1][k0|k1][q2|k2];
                                    bias folded into the PSUM->SBUF cast)
  - S^T        = K^T-chunks vs Q^T (keys on partitions, queries on free dim)
  - P~         = exp(0.125 * S^T)  (no max-subtraction; logits are tiny)
  - U^T        = [V|1].T @ P~      (ones-column gives softmax denominator row)
  - O^T        = U^T * (1/den)     (recip on DVE, bcast on GpSimd, mult DVE)
  - Z^T        = Wproj-slice @ O^T (emitted per query-pair, overlaps attn)
V-bias commutes through softmax (O = PV/den + bv), so it is folded into
proj_b on the host: out += proj_b + proj_w @ bv.
Host sums the 4 head-group partials per batch.
"""

import functools

import ml_dtypes
import numpy as np

import concourse.bass as bass
import concourse.bacc as bacc_mod
import concourse.mybir as mybir
import concourse.tile as tile
from concourse.bass import ts

F32 = mybir.dt.float32
F32R = mybir.dt.float32r
BF16 = mybir.dt.bfloat16

B, T, N, D = 2, 8, 256, 768
H, HD = 12, 64
L = T * N          # 2048
HPC = 3            # heads per core
NKC = L // 128     # 16 key chunks of 128
NDC = D // 128     # 6 contraction chunks
NNT = L // 512     # 4 tiles of 512 along L
SCALE = 1.0 / 8.0


def build_nc():
    nc = bacc_mod.Bacc()

    xT_d = nc.declare_dram_parameter("xT", [D, L], BF16, isOutput=False)
    wqkT_d = nc.declare_dram_parameter("wqkT", [D, 384], BF16, isOutput=False)
    wvnat_d = nc.declare_dram_parameter("wvnat", [D, 192], BF16, isOutput=False)
    bqk_d = nc.declare_dram_parameter("bqk", [128, 3], F32, isOutput=False)
    wprojT_d = nc.declare_dram_parameter("wprojT", [128, 1536], BF16, isOutput=False)
    zT_d = nc.declare_dram_parameter("zT", [D, L], BF16, isOutput=True)

    ACT = mybir.ActivationFunctionType

    with tile.TileContext(nc) as tc:
        with (
            tc.tile_pool(name="persist", bufs=1) as pp,
            tc.tile_pool(name="ptile", bufs=18) as ppool,
            tc.tile_pool(name="small", bufs=6) as spool,
            tc.tile_pool(name="zbuf", bufs=3) as zpool,
            tc.tile_pool(name="psum_q", bufs=2, space="PSUM") as pq,
            tc.tile_pool(name="psum_st", bufs=2, space="PSUM") as pst,
            tc.tile_pool(name="psum_ut", bufs=2, space="PSUM") as put,
        ):
            # ---- persistent SBUF tensors ----
            xT = pp.tile([128, NDC, L], BF16, tag="xT")
            wqkT = pp.tile([128, NDC, 384], BF16, tag="wqkT")
            wvnat = pp.tile([128, NDC, 192], BF16, tag="wvnat")
            bqk = pp.tile([128, 3], F32, tag="bqk")
            wprojT = pp.tile([128, 1536], BF16, tag="wprojT")
            # q/k transposed activations: rows are head dims
            qkA = pp.tile([128, L], BF16, tag="qkA")    # [q0 | q1]
            qkB = pp.tile([128, L], BF16, tag="qkB")    # [k0 | k1]
            qkC = pp.tile([128, L], BF16, tag="qkC")    # [q2 | --]
            kt2 = pp.tile([64, L], BF16, tag="kt2")     # k2 re-based to part 0
            # natural-layout V, augmented with a ones column per head
            vn3 = pp.tile([128, NKC, HPC, 65], BF16, tag="vn3")
            # normalized attention outputs (transposed): rows are head dims
            otp = pp.tile([128, L], BF16, tag="otp")    # [o0 | o1]
            ots = pp.tile([64, L], BF16, tag="ots")     # [o2]

            # ---- input DMAs, batched (DMA issue costs ~600ns each on a
            #      sequencer, so few big transfers + spread across engines) ----
            nc.gpsimd.dma_start(
                out=wvnat[:], in_=wvnat_d[:, :].rearrange("(dc p) c -> p dc c", p=128)
            )
            nc.gpsimd.dma_start(out=bqk[:], in_=bqk_d[:, :])
            nc.scalar.dma_start(
                out=wqkT[:], in_=wqkT_d[:, :].rearrange("(dc p) c -> p dc c", p=128)
            )
            nc.scalar.dma_start(out=wprojT[:], in_=wprojT_d[:, :])
            xT_v = xT_d[:, :].rearrange("(dc p) l -> p dc l", p=128)
            # nt0 arrives as four 128-column chunks so the first V-nat matmul
            # only waits for ~190KB; later nt tiles stream behind it.
            for kc in range(4):
                eng = nc.sync if kc % 2 == 0 else nc.scalar
                eng.dma_start(
                    out=xT[:, :, ts(kc, 128)], in_=xT_v[:, :, ts(kc, 128)]
                )
            nc.sync.dma_start(out=xT[:, :, ts(1, 512)], in_=xT_v[:, :, ts(1, 512)])
            nc.gpsimd.dma_start(out=xT[:, :, ts(2, 512)], in_=xT_v[:, :, ts(2, 512)])
            nc.scalar.dma_start(out=xT[:, :, ts(3, 512)], in_=xT_v[:, :, ts(3, 512)])

            # ones columns of vn3 (written once; V copies fill cols 0:64)
            nc.vector.memset(vn3[:, :, :, 64], 1.0)

            # ---- phase 1 emitters: V natural per key chunk, Q^T/K^T chunks.
            #      kc 8-15 of V-nat and the q2|k2 chunk are deferred into the
            #      first attention block as PE filler (keeps the PE busy while
            #      the Act engine works through the first strip's exps). ----
            def emit_vnat(kc):
                pv = pq.tile([128, 512], F32, tag="pq")
                for dc in range(NDC):
                    nc.tensor.matmul(
                        pv[:, 0:192],
                        xT[:, dc, ts(kc, 128)],
                        wvnat[:, dc, :],
                        start=(dc == 0),
                        stop=(dc == NDC - 1),
                    )
                nc.vector.tensor_copy(
                    vn3[:, kc, :, 0:64],
                    pv[:, 0:192].rearrange("p (h d) -> p h d", d=64),
                )

            mc_dst = [qkA, qkB, qkC]

            def emit_qk(mc, nt):
                ps = pq.tile([128, 512], F32, tag="pq")
                for dc in range(NDC):
                    nc.tensor.matmul(
                        ps[:],
                        wqkT[:, dc, ts(mc, 128)],
                        xT[:, dc, ts(nt, 512)],
                        start=(dc == 0),
                        stop=(dc == NDC - 1),
                    )
                if mc < 2:
                    # full-chunk cast + per-partition bias add
                    nc.vector.tensor_scalar(
                        mc_dst[mc][:, ts(nt, 512)],
                        ps[:],
                        bqk[:, mc : mc + 1],
                        None,
                        op0=mybir.AluOpType.add,
                    )
                else:
                    # q2 stays at partitions 0:64; k2 shifts to 0:64 (DVE
                    # ops may move partition base; PE operands may not)
                    nc.vector.tensor_scalar(
                        qkC[0:64, ts(nt, 512)],
                        ps[0:64, :],
                        bqk[0:64, 2:3],
                        None,
                        op0=mybir.AluOpType.add,
                    )
                    nc.vector.tensor_scalar(
                        kt2[0:64, ts(nt, 512)],
                        ps[64:128, :],
                        bqk[64:128, 2:3],
                        None,
                        op0=mybir.AluOpType.add,
                    )

            for kc in range(8):
                emit_vnat(kc)
            for mc in range(2):
                for nt in range(NNT):
                    emit_qk(mc, nt)
            filler = [lambda kc=kc: emit_vnat(kc) for kc in range(8, NKC)]
            filler += [lambda nt=nt: emit_qk(2, nt) for nt in range(NNT)]

            # pre-warm the exp activation table before the first real exp
            warm = spool.tile([1, 32], F32, tag="warm")
            nc.vector.memset(warm[:], 0.0)
            nc.scalar.activation(warm[:], warm[:], ACT.Exp)

            # ---- phase 2: attention, query-pair outer / head inner;
            #      normalize is emitted one strip late so the PE never waits
            #      on the DVE recip chain; proj for a pair follows its h2
            #      normalize and fills PE gaps in the next pair's strips ----
            qsrc = [qkA[0:64, :], qkA[64:128, :], qkC[0:64, :]]
            ksrc = [qkB[0:64, :], qkB[64:128, :], kt2[0:64, :]]
            ot_dst = [otp[0:64, :], otp[64:128, :], ots[0:64, :]]

            def emit_normalize(qq, h, ut):
                # O^T = U^T * (1/den); den = ones row of U^T.  1/den is
                # broadcast across 64 partitions with stream_shuffle; the
                # whole chain stays off the PE.
                q_lo = 512 * qq
                den = spool.tile([1, 512], F32, tag="den")
                bcs = spool.tile([64, 512], F32, tag="bcs")
                nc.vector.tensor_copy(den[:], ut[64:65, :])
                nc.vector.reciprocal_approx_fast(bcs[0:1, :], den[:])
                nc.vector.stream_shuffle(bcs[0:32, :], bcs[0:32, :], [0] * 32)
                nc.vector.stream_shuffle(bcs[32:64, :], bcs[0:32, :], [0] * 32)
                nc.vector.tensor_tensor(
                    out=ot_dst[h][:, q_lo : q_lo + 512],
                    in0=ut[0:64, :],
                    in1=bcs[:],
                    op=mybir.AluOpType.mult,
                )

            zT_v = zT_d[:, :].rearrange("(m p) l -> p m l", p=128)

            def emit_proj(qq):
                for half in range(2):
                    zb = zpool.tile([128, 3, 512], BF16, tag="zb")
                    for j in range(3):
                        mc = 3 * half + j
                        ps = pq.tile([128, 512], F32, tag="pq")
                        nc.tensor.matmul(
                            ps[:],
                            wprojT[:, ts(mc, 128)],
                            otp[:, ts(qq, 512)],
                            start=True,
                            stop=False,
                        )
                        nc.tensor.matmul(
                            ps[:],
                            wprojT[0:64, 768 + mc * 128 : 768 + (mc + 1) * 128],
                            ots[0:64, ts(qq, 512)],
                            start=False,
                            stop=True,
                        )
                        if qq < 2:
                            nc.vector.tensor_copy(zb[:, j, :], ps[:])
                        else:
                            nc.scalar.copy(zb[:, j, :], ps[:])
                    nc.sync.dma_start(
                        out=zT_v[:, 3 * half : 3 * half + 3, ts(qq, 512)],
                        in_=zb[:],
                    )

            # Strip-level software pipeline: block i emits strip i's
            # S+exp work interleaved with strip i-1's PV matmuls, so a PV
            # always consumes an exp that finished a whole block ago and the
            # PE never stalls on the Act engine (a stalled PE re-throttles
            # the clock and runs at half speed for ~3us).  Block 0 has no PV
            # partner, so the deferred V-nat / q2k2 chunks fill it instead.
            strips = [(qq, h) for qq in range(NNT) for h in range(HPC)]

            def strip_kbs(qq):
                return list(range(2 * qq + 1, T)) + [2 * qq]

            prev = None  # (qq, h, kbs, pts, ut)
            for i in range(len(strips) + 1):
                cur = None
                if i < len(strips):
                    qq, h = strips[i]
                    cur = (qq, h, strip_kbs(qq), [], None)
                nsteps = max(len(cur[2]) if cur else 0,
                             len(prev[2]) if prev else 0)
                fill_per = (len(filler) + nsteps - 1) // nsteps if i == 0 else 0
                for j in range(nsteps):
                    if cur and j < len(cur[2]):
                        qq, h, kbs, pts, _ = cur
                        kb = kbs[j]
                        qb0 = 2 * qq
                        q_lo = 512 * qq
                        seg = 512 if kb > qb0 else 256
                        st = pst.tile([128, 1024], F32, tag="st")
                        pt = ppool.tile([128, 1024], BF16, tag="pt")
                        for half in range(2):
                            nc.tensor.matmul(
                                st[:, half * seg : half * seg + seg],
                                ksrc[h][:, ts(2 * kb + half, 128)],
                                qsrc[h][:, q_lo : q_lo + seg],
                                start=True,
                                stop=True,
                            )
                        nc.scalar.activation(
                            pt[:, 0 : 2 * seg],
                            st[:, 0 : 2 * seg],
                            ACT.Exp,
                            scale=SCALE,
                        )
                        pts.append(pt)
                    if i == 0:
                        for _ in range(fill_per):
                            if filler:
                                filler.pop(0)()
                    if prev and j < len(prev[2]):
                        qq_p, h_p, kbs_p, pts_p, ut_p = prev
                        if ut_p is None:
                            ut_p = put.tile([128, 512], F32, tag="ut")
                            prev = (qq_p, h_p, kbs_p, pts_p, ut_p)
                        kb = kbs_p[j]
                        qb0 = 2 * qq_p
                        seg = 512 if kb > qb0 else 256
                        pt = pts_p[j]
                        for half in range(2):
                            nc.tensor.matmul(
                                ut_p[0:65, 0:seg],
                                vn3[:, 2 * kb + half, h_p, :],
                                pt[:, half * seg : half * seg + seg],
                                start=(j == 0 and half == 0),
                                stop=(j == len(kbs_p) - 1 and half == 1),
                                skip_group_check=True,
                            )
                if prev:
                    qq_p, h_p, _, _, ut_p = prev
                    emit_normalize(qq_p, h_p, ut_p)
                    if h_p == HPC - 1:
                        emit_proj(qq_p)
                prev = cur

    nc.compile()
    return nc


@functools.lru_cache(maxsize=1)
def get_nc():
    return build_nc()


def make_in_maps(x, qkv_w, qkv_b, proj_w):
    """Per-core host-side sharding/layout prep."""
    x = np.asarray(x, dtype=np.float32)
    qkv_w = np.asarray(qkv_w, dtype=np.float32)
    qkv_b = np.asarray(qkv_b, dtype=np.float32)
    proj_w = np.asarray(proj_w, dtype=np.float32)

    in_maps = []
    for c in range(8):
        b, g = divmod(c, 4)
        heads = [3 * g, 3 * g + 1, 3 * g + 2]

        def qrows(h):
            return slice(h * HD, (h + 1) * HD)

        def krows(h):
            return slice(D + h * HD, D + (h + 1) * HD)

        def vrows(h):
            return slice(2 * D + h * HD, 2 * D + (h + 1) * HD)

        h0, h1, h2 = heads
        # [q0|q1] [k0|k1] [q2|k2]  (384, 768)
        order = [qrows(h0), qrows(h1), krows(h0), krows(h1), qrows(h2), krows(h2)]
        wsel = np.concatenate([qkv_w[s] for s in order], axis=0)
        bsel = np.concatenate([qkv_b[s] for s in order], axis=0)
        bqk = np.ascontiguousarray(bsel.reshape(3, 128).T)           # (128, 3)
        # V natural weights: (768, 192), col h*64+d = Wv_h[d]
        wvn = np.concatenate([qkv_w[vrows(h)] for h in heads], axis=0).T
        wpp = np.concatenate(
            [proj_w[:, ts_np(h0)].T, proj_w[:, ts_np(h1)].T], axis=0
        )  # (128, 768)
        wps = np.concatenate(
            [proj_w[:, ts_np(h2)].T, np.zeros((64, D), np.float32)], axis=0
        )  # (128, 768)
        in_maps.append(
            {
                "xT": np.ascontiguousarray(x[b].reshape(L, D).T).astype(
                    ml_dtypes.bfloat16
                ),
                "wqkT": np.ascontiguousarray(wsel.T).astype(ml_dtypes.bfloat16),
                "wvnat": np.ascontiguousarray(wvn).astype(ml_dtypes.bfloat16),
                "bqk": bqk,
                "wprojT": np.ascontiguousarray(
                    np.concatenate([wpp, wps], axis=1)
                ).astype(ml_dtypes.bfloat16),
            }
        )
    return in_maps


def ts_np(h):
    return slice(h * HD, (h + 1) * HD)


def assemble_output(results, qkv_b, proj_w, proj_b):
    qkv_b = np.asarray(qkv_b, dtype=np.float32)
    proj_w = np.asarray(proj_w, dtype=np.float32)
    proj_b = np.asarray(proj_b, dtype=np.float32)
    # v-bias commutes through the softmax normalization: O = PV/den + bv
    pb2 = proj_b + proj_w @ qkv_b[2 * D :]
    out = np.zeros((B, L, D), np.float32)
    for c in range(8):
        b = c // 4
        out[b] += results[c]["zT"].astype(np.float32).T
    out += pb2[None, None, :]
    return out.reshape(B, T, N, D)


def _install_ntff_hook():
    """The container's antenv stub lacks axon_hooks; recreate it from the
    boot helper so trace=True can profile through libaxon_pjrt."""
    import sys
    import types

    try:
        from antenv.axon_hooks import get_axon_ntff_profile_hook  # noqa: F401

        return
    except ImportError:
        pass
    import antenv
    from trn_agent_boot.trn_boot import _ntff_profile_via_ctypes

    state = {"hook": _ntff_profile_via_ctypes("/opt/axon/libaxon_pjrt.so")}
    mod = types.ModuleType("antenv.axon_hooks")
    mod.set_axon_ntff_profile_hook = lambda h: state.__setitem__("hook", h)
    mod.get_axon_ntff_profile_hook = lambda: state["hook"]
    sys.modules["antenv.axon_hooks"] = mod
    antenv.axon_hooks = mod

    import concourse.bass_utils as bu

    orig_upload = bu.upload_artifacts

    def safe_upload(tmpdir):
        try:
            return orig_upload(tmpdir)
        except Exception:
            return tmpdir

    bu.upload_artifacts = safe_upload


def kernel_with_stats(x, qkv_w, qkv_b, proj_w, proj_b, trace=False):
    from concourse.bass_utils import run_bass_kernel_spmd

    if trace:
        _install_ntff_hook()
    nc = get_nc()
    in_maps = make_in_maps(x, qkv_w, qkv_b, proj_w)
    res = run_bass_kernel_spmd(nc, in_maps, list(range(8)), trace=trace)
    return assemble_output(res.results, qkv_b, proj_w, proj_b), res


def kernel(x, qkv_w, qkv_b, proj_w, proj_b):
    out, _ = kernel_with_stats(x, qkv_w, qkv_b, proj_w, proj_b)
    return out
